# revision 6
# baseline (speedup 1.0000x reference)
"""MultiHeadAttention (B=1, S=4096, D=768, H=12) on 8 Trainium2 NeuronCores.

Sharding: core pair j=c//2 owns heads 3j..3j+2 (192 e-cols); even cores
compute queries 0..2047, odd cores 2048..4095.  Each core projects K/V for
its 3 heads over the full sequence (duplicated x2 within a pair), Q for its
q-half, runs attention in S^T orientation (softmax denominator via an
appended ones-column in the V matmul), and emits a partial output
(ctx_slice @ wo_cols^T).  Host sums the 4 head-triple partials per q-half
and adds wo_b.  All weight transposes are done host-side with numpy.
"""

import sys

sys.path.insert(0, "/opt/trn_rl_repo")

import numpy as np

import concourse.bass as bass  # noqa: F401
import concourse.tile as tile
import concourse.mybir as mybir
from concourse import bacc, bass_utils

P = 128
D = 768
DC = D // P  # 6 contraction chunks
S = 4096
SCH = S // 512  # 8 sequence chunks for K/V projection
SKT = S // P  # 32 k-tiles
QN = 2048  # queries per core
QCH = QN // 512  # 4 q-chunks per core
HPC = 3  # heads per core
E3 = HPC * 64  # 192 e-cols per core
NCORES = 8
F32 = mybir.dt.float32
F32R = mybir.dt.float32r
EXPF = mybir.ActivationFunctionType.Exp


def _emit(tc, io):
    nc = tc.nc
    import contextlib

    ctx = contextlib.ExitStack()
    with ctx:
        singles = ctx.enter_context(tc.tile_pool(name="singles", bufs=1))
        xs = ctx.enter_context(tc.tile_pool(name="xs", bufs=3))
        pp = ctx.enter_context(tc.tile_pool(name="pp", bufs=3))
        smalls = ctx.enter_context(tc.tile_pool(name="smalls", bufs=2))
        outp = ctx.enter_context(tc.tile_pool(name="outp", bufs=3))
        spsum = ctx.enter_context(tc.tile_pool(name="spsum", bufs=2, space="PSUM"))
        upsum = ctx.enter_context(tc.tile_pool(name="upsum", bufs=2, space="PSUM"))

        # ---- constants / weights ----
        wq_sb = singles.tile([P, DC, E3], F32R)
        wk_sb = singles.tile([P, DC, E3], F32R)
        wv_sb = singles.tile([P, DC, E3], F32R)
        for t, a in ((wq_sb, io["wqT"]), (wk_sb, io["wkT"]), (wv_sb, io["wvT"])):
            nc.sync.dma_start(t[:], a.rearrange("(dc p) e -> p dc e", p=P))
        wo1_sb = singles.tile([P, D], F32R)
        nc.sync.dma_start(wo1_sb[:], io["wo1"])
        wo2_sb = singles.tile([64, D], F32R)
        nc.sync.dma_start(wo2_sb[:], io["wo2"])
        qb1 = singles.tile([P, 1], F32)
        nc.sync.dma_start(qb1[:], io["qb"][0:P, :])
        qb2 = singles.tile([64, 1], F32)
        nc.sync.dma_start(qb2[:], io["qb"][P:E3, :])
        kb1 = singles.tile([P, 1], F32)
        nc.sync.dma_start(kb1[:], io["kb"][0:P, :])
        kb2 = singles.tile([64, 1], F32)
        nc.sync.dma_start(kb2[:], io["kb"][P:E3, :])
        vb_sb = singles.tile([P, HPC, 64], F32)
        nc.sync.dma_start(vb_sb[:], io["vb"].rearrange("p (h d) -> p h d", h=HPC))
        ones1 = singles.tile([1, 64], F32R)
        nc.sync.dma_start(ones1[:], io["ones"][0:1, 0:64])

        # ---- persistent activations ----
        KT1 = singles.tile([P, S], F32R)  # K^T rows: head0 d 0-63, head1 d 64-127
        KT2 = singles.tile([64, S], F32R)  # head2
        QT1 = singles.tile([P, QN], F32R)
        QT2 = singles.tile([64, QN], F32R)
        VA = singles.tile([P, SKT, HPC, 65], F32R)  # [V | ones] per k-tile/head
        CT1 = singles.tile([P, QN], F32R)  # ctx^T rows: head0 0-63, head1 64-127
        CT2 = singles.tile([64, QN], F32R)
        nc.sync.dma_start(
            VA[:, :, :, 64:65],
            io["ones"].rearrange("p (a b one) -> p a b one", a=SKT, b=HPC, one=1),
        )  # pre-set ones columns (col 64)

        # ---- phase 1: K^T and V projections over full sequence ----
        for sc in range(SCH):
            xt = xs.tile([P, DC, 512], F32R, tag="xs")
            nc.sync.dma_start(
                xt[:],
                io["xT"][:, sc * 512 : (sc + 1) * 512].rearrange(
                    "(dc p) s -> p dc s", p=P
                ),
            )
            for dst, c0, m, kb_t in ((KT1, 0, P, kb1), (KT2, P, 64, kb2)):
                ps = upsum.tile([P, 512], F32, tag="u")
                for dc in range(DC):
                    nc.tensor.matmul(
                        ps[:m],
                        (wk_sb[:, dc, c0 : c0 + m]),
                        (xt[:, dc, :]),
                        start=(dc == 0),
                        stop=(dc == DC - 1),
                    )
                nc.vector.tensor_add(
                    out=dst[:m, sc * 512 : (sc + 1) * 512],
                    in0=ps[:m],
                    in1=kb_t[:].to_broadcast((m, 512)),
                )
            for ss in range(4):
                kt = sc * 4 + ss
                ps = upsum.tile([P, 512], F32, tag="u")
                for dc in range(DC):
                    nc.tensor.matmul(
                        ps[:, :E3],
                        (xt[:, dc, ss * P : (ss + 1) * P]),
                        (wv_sb[:, dc, :]),
                        start=(dc == 0),
                        stop=(dc == DC - 1),
                    )
                nc.vector.tensor_add(
                    out=VA[:, kt, :, 0:64],
                    in0=ps[:, :E3].rearrange("p (h d) -> p h d", h=HPC),
                    in1=vb_sb[:],
                )

        # ---- phase 2: Q^T projection for this core's q-half ----
        for qsc in range(QCH):
            xt = xs.tile([P, DC, 512], F32R, tag="xs")
            nc.sync.dma_start(
                xt[:],
                io["xqT"][:, qsc * 512 : (qsc + 1) * 512].rearrange(
                    "(dc p) s -> p dc s", p=P
                ),
            )
            for dst, c0, m, qb_t in ((QT1, 0, P, qb1), (QT2, P, 64, qb2)):
                ps = upsum.tile([P, 512], F32, tag="u")
                for dc in range(DC):
                    nc.tensor.matmul(
                        ps[:m],
                        (wq_sb[:, dc, c0 : c0 + m]),
                        (xt[:, dc, :]),
                        start=(dc == 0),
                        stop=(dc == DC - 1),
                    )
                nc.vector.tensor_add(
                    out=dst[:m, qsc * 512 : (qsc + 1) * 512],
                    in0=ps[:m],
                    in1=qb_t[:].to_broadcast((m, 512)),
                )

        # ---- phase 3: attention, S^T orientation ----
        def kt_src(h):
            return (KT1, 64 * h) if h < 2 else (KT2, 0)

        def qt_src(h):
            return (QT1, 64 * h) if h < 2 else (QT2, 0)

        def attn_pass(qc, heads):
            nh = len(heads)
            nslots = SKT * nh
            us = [
                upsum.tile([P, 512], F32, tag="u", name=f"u_{hi}") for hi in range(nh)
            ]
            ngroups = (nslots + 2) // 3
            for g in range(ngroups):
                w = min(3, nslots - g * 3)
                sg = spsum.tile([P, 1536], F32, tag="s")
                for i in range(w):
                    s = g * 3 + i
                    kt, hi = s // nh, s % nh
                    KT, kp = kt_src(heads[hi])
                    QT, qp = qt_src(heads[hi])
                    nc.tensor.matmul(
                        sg[:, i * 512 : (i + 1) * 512],
                        (KT[kp : kp + 64, kt * P : (kt + 1) * P]),
                        (QT[qp : qp + 64, qc * 512 : (qc + 1) * 512]),
                        start=True,
                        stop=True,
                    )
                pg = pp.tile([P, 1536], F32R, tag="p")
                nc.scalar.activation(
                    out=pg[:, : w * 512], in_=sg[:, : w * 512], func=EXPF, scale=0.125
                )
                for i in range(w):
                    s = g * 3 + i
                    kt, hi = s // nh, s % nh
                    nc.tensor.matmul(
                        us[hi][:65],
                        (VA[:, kt, heads[hi], :]),
                        (pg[:, i * 512 : (i + 1) * 512]),
                        start=(kt == 0),
                        stop=(kt == SKT - 1),
                    )
            for hi, h in enumerate(heads):
                rz = smalls.tile([1, 512], F32R, tag="rz")
                with nc.allow_low_precision(reason="1/Z rounded to fp22 for PE rhs"):
                    nc.vector.reciprocal(out=rz[:], in_=us[hi][64:65, :])
                zb_ps = spsum.tile([64, 512], F32, tag="s")
                nc.tensor.matmul(zb_ps[:], (ones1[:]), (rz[:]), start=True, stop=True)
                zb = smalls.tile([64, 512], F32, tag="zb")
                nc.vector.tensor_copy(out=zb[:], in_=zb_ps[:])
                CT, cp = (CT1, 64 * h) if h < 2 else (CT2, 0)
                nc.vector.tensor_mul(
                    out=CT[cp : cp + 64, qc * 512 : (qc + 1) * 512],
                    in0=us[hi][0:64, :],
                    in1=zb[:],
                )

        for qc in range(QCH):
            attn_pass(qc, [0, 1])
            attn_pass(qc, [2])

        # ---- phase 4: partial output projection ----
        for qs in range(QN // P):
            ob = outp.tile([P, D], F32, tag="ob")
            for n0, nw in ((0, 512), (512, 256)):
                ps = upsum.tile([P, 512], F32, tag="u")
                nc.tensor.matmul(
                    ps[:, :nw],
                    (CT1[:, qs * P : (qs + 1) * P]),
                    (wo1_sb[:, n0 : n0 + nw]),
                    start=True,
                    stop=False,
                )
                nc.tensor.matmul(
                    ps[:, :nw],
                    (CT2[:, qs * P : (qs + 1) * P]),
                    (wo2_sb[:, n0 : n0 + nw]),
                    start=False,
                    stop=True,
                )
                nc.vector.tensor_copy(out=ob[:, n0 : n0 + nw], in_=ps[:, :nw])
            nc.sync.dma_start(io["out"][qs * P : (qs + 1) * P, :], ob[:])


def _build():
    nc = bacc.Bacc("TRN2", target_bir_lowering=False, debug=False, num_devices=NCORES)
    io = {}
    for name, shape, dt in (
        ("xT", [D, S], F32R),
        ("xqT", [D, QN], F32R),
        ("wqT", [D, E3], F32R),
        ("wkT", [D, E3], F32R),
        ("wvT", [D, E3], F32R),
        ("wo1", [P, D], F32R),
        ("wo2", [64, D], F32R),
        ("qb", [E3, 1], F32),
        ("kb", [E3, 1], F32),
        ("vb", [P, E3], F32),
        ("ones", [P, SKT * HPC], F32R),
    ):
        io[name] = nc.dram_tensor(name, shape, dt, kind="ExternalInput").ap()
    io["out"] = nc.dram_tensor("out", [QN, D], F32, kind="ExternalOutput").ap()
    with tile.TileContext(nc) as tc:
        _emit(tc, io)
    nc.compile()
    return nc


_CACHE = {}


def _get_nc():
    if "nc" not in _CACHE:
        _CACHE["nc"] = _build()
    return _CACHE["nc"]


def make_in_maps(x, wq_w, wq_b, wk_w, wk_b, wv_w, wv_b, wo_w, wo_b):
    xT = np.ascontiguousarray(x[0].T)  # [768, 4096]
    in_maps = []
    for c in range(NCORES):
        j = c // 2
        c0 = E3 * j
        cols = slice(c0, c0 + E3)
        rows = slice(0, QN) if c % 2 == 0 else slice(QN, S)
        in_maps.append(
            {
                "xT": xT,
                "xqT": np.ascontiguousarray(xT[:, rows]),
                "wqT": np.ascontiguousarray(wq_w[cols, :].T),
                "wkT": np.ascontiguousarray(wk_w[cols, :].T),
                "wvT": np.ascontiguousarray(wv_w[cols, :].T),
                "wo1": np.ascontiguousarray(wo_w[:, c0 : c0 + P].T),
                "wo2": np.ascontiguousarray(wo_w[:, c0 + P : c0 + E3].T),
                "qb": np.ascontiguousarray(wq_b[cols].reshape(E3, 1)),
                "kb": np.ascontiguousarray(wk_b[cols].reshape(E3, 1)),
                "vb": np.ascontiguousarray(
                    np.broadcast_to(wv_b[cols], (P, E3)).copy()
                ),
                "ones": np.ones((P, SKT * HPC), np.float32),
            }
        )
    return in_maps


def assemble(results, wo_b):
    out = np.zeros((S, D), np.float32)
    for c in range(NCORES):
        rows = slice(0, QN) if c % 2 == 0 else slice(QN, S)
        out[rows] += results[c]["out"]
    out += wo_b
    return out[None]


def kernel(**inputs):
    a = {k: np.asarray(v, np.float32) for k, v in inputs.items()}
    nc = _get_nc()
    in_maps = make_in_maps(
        a["x"], a["wq_w"], a["wq_b"], a["wk_w"], a["wk_b"],
        a["wv_w"], a["wv_b"], a["wo_w"], a["wo_b"],
    )
    res = bass_utils.run_bass_kernel_spmd(nc, in_maps, core_ids=list(range(NCORES)))
    _CACHE["last_results"] = res
    return assemble(res.results, a["wo_b"])
